# revision 57
# baseline (speedup 1.0000x reference)
"""MoE expert-parallel kernel for Trainium2 (8 NeuronCores).

Problem: nn_DistributedExpertPool — each of 2048 tokens (H=1024) is routed to
one of 8 experts; expert e applies Linear(H->F=2048) -> exact GELU ->
Linear(F->H).

Strategy (expert parallelism, matching the sharding hint):
  - Host: sort tokens by expert assignment ("dispatch"), pad each expert's
    token batch to a common capacity CAP (multiple of 128), and pre-transpose
    to x.T layout [H, CAP] so the device kernel only ever streams K-major
    operands.
  - Core c gets expert c's weights (W1[c] [H,F], W2[c] [F,H], biases) plus its
    token batch. Device computes y.T = W2.T @ gelu(W1.T @ x.T + b1) + b2
    entirely on-chip (weights resident in SBUF, PSUM accumulation over K).
  - Host: scatter each core's outputs back to the original token order
    ("combine").

The device kernel keeps both matmuls in the transposed layout so the GELU
bias (b1, per-F) and the output bias (b2, per-H) are per-partition vectors,
which the ScalarE activation op applies for free.

Matmul operands stream as fp16 (weights are ~N(0, 0.02), activations O(1) —
well inside fp16 range; 10-bit mantissa beats bf16 by 8x here). PSUM
accumulation stays fp32 and the output is stored fp32. Measured 4.2e-4
relative error end-to-end vs the fp32 reference. KM_MMDT=fp32r (2.1e-4,
~1.5x slower) and KM_MMDT=fp32 (4e-7, ~5x slower) are exactness fallbacks.
"""

import os as _os
import sys as _sys

import numpy as np

try:
    import concourse.bass as bass
except ImportError:  # fresh dirs without the site hook on sys.path
    for _p in ("/opt/trn_rl_repo", "/root/.axon_site/_ro/trn_rl_repo"):
        if _p not in _sys.path:
            _sys.path.append(_p)
    import concourse.bass as bass  # noqa: E402
import concourse.tile as tile
from concourse import mybir
from concourse.bass_utils import run_bass_kernel_spmd  # noqa: F401 (fallback)

_jit_cache: dict[int, tuple] = {}


def _run_spmd_cached(nc, in_maps):
    """run_bass_kernel_spmd's axon/PJRT path with the jitted executable cached
    per program — the concourse shim rebuilds its jax.jit closure every call,
    paying ~1.5s of retrace; reusing one function object makes repeat calls
    dispatch in milliseconds."""
    import jax
    import numpy as _np
    from jax.sharding import Mesh, PartitionSpec
    from jax.experimental.shard_map import shard_map
    from concourse import bass2jax, mybir as _mb

    key = id(nc)
    if key not in _jit_cache:
        bass2jax.install_neuronx_cc_hook()
        partition_name = (nc.partition_id_tensor.name
                          if nc.partition_id_tensor else None)
        in_names, out_names, out_avals = [], [], []
        for alloc in nc.m.functions[0].allocations:
            if not isinstance(alloc, _mb.MemoryLocationSet):
                continue
            name = alloc.memorylocations[0].name
            if alloc.kind == "ExternalInput":
                if name != partition_name:
                    in_names.append(name)
            elif alloc.kind == "ExternalOutput":
                out_names.append(name)
                out_avals.append(jax.core.ShapedArray(
                    tuple(alloc.tensor_shape), _mb.dt.np(alloc.dtype)))
        n_params = len(in_names)
        all_names = list(in_names) + list(out_names)
        if partition_name is not None:
            all_names.append(partition_name)

        def _body(*args):
            operands = list(args)
            if partition_name is not None:
                operands.append(bass2jax.partition_id_tensor())
            return tuple(bass2jax._bass_exec_p.bind(
                *operands, out_avals=tuple(out_avals),
                in_names=tuple(all_names), out_names=tuple(out_names),
                lowering_input_output_aliases=(),
                sim_require_finite=True, sim_require_nnan=True, nc=nc))

        devices = jax.devices()[:N_CORES]
        mesh = Mesh(_np.asarray(devices), ("core",))
        n_outs = len(out_names)
        sharded = jax.jit(
            shard_map(_body, mesh=mesh,
                      in_specs=(PartitionSpec("core"),) * (n_params + n_outs),
                      out_specs=(PartitionSpec("core"),) * n_outs,
                      check_rep=False),
            donate_argnums=tuple(range(n_params, n_params + n_outs)),
            keep_unused=True)
        _jit_cache[key] = (sharded, in_names, out_names, out_avals, n_params)

    sharded, in_names, out_names, out_avals, n_params = _jit_cache[key]
    concat_in = [
        _np.concatenate([_np.asarray(m[name]) for m in in_maps], axis=0)
        for name in in_names]
    concat_zeros = [
        _np.zeros((N_CORES * a.shape[0], *a.shape[1:]), a.dtype)
        for a in out_avals]
    out_arrs = sharded(*concat_in, *concat_zeros)

    class _R:
        results = [
            {name: _np.asarray(out_arrs[i]).reshape(
                N_CORES, *out_avals[i].shape)[c]
             for i, name in enumerate(out_names)}
            for c in range(N_CORES)]
    return _R()

TOKENS = 2048
HIDDEN = 1024
FFN = 2048
NUM_EXPERTS = 8
N_CORES = 8

KH = HIDDEN // 128  # 8 K-tiles for the first matmul
KF = FFN // 128     # 16 K-tiles for the second matmul

_compiled_cache: dict[tuple, bass.Bass] = {}

# PE streaming dtype for matmul operands: float32 = exact two-pass (4 cyc/row),
# float32r = single-pass reduced-precision (1 cyc/row at N>=256).
MM_DTYPE = {"fp32": mybir.dt.float32, "fp32r": mybir.dt.float32r,
            "fp16": mybir.dt.float16}[_os.environ.get("KM_MMDT", "fp16")]


def _split_multi_waits(nc: bass.Bass) -> None:
    """Walrus in this toolchain accepts at most ONE sync-wait per instruction
    ("Too many sync wait commands" in setupSyncWait otherwise). Tile's
    scheduler happily attaches several. Split the extras into NoOps placed
    just before the instruction on the same engine queue — the NX sequencer
    processes them in order, so the semantics are identical."""
    for fn in nc.m.functions:
        for blk in fn.blocks:
            out = []
            changed = False
            for inst in blk.instructions:
                si = inst.sync_info
                if si is not None and si.on_wait is not None and len(si.on_wait) > 1:
                    waits = list(si.on_wait)
                    for j, w in enumerate(waits[:-1]):
                        nop = mybir.InstNoOp(
                            name=f"{inst.name}-wsplit{j}", ins=[], outs=[])
                        nop.engine = inst.engine
                        nop.sync_info = mybir.SyncInfo(on_wait=[w], on_update=[])
                        out.append(nop)
                    inst.sync_info = mybir.SyncInfo(
                        on_wait=[waits[-1]],
                        on_update=list(si.on_update) if si.on_update else [],
                    )
                    changed = True
                out.append(inst)
            if changed:
                blk.instructions = out


def _build_nc(cap: int, mm_dtype=None) -> bass.Bass:
    """Build the per-core Bass program for token capacity `cap` (mult of 128)."""
    fp32 = mybir.dt.float32
    mmdt = MM_DTYPE if mm_dtype is None else mm_dtype
    nc = bass.Bass("TRN2", target_bir_lowering=False, debug=False,
                   num_devices=N_CORES)

    # All streaming operands are host-preswizzled into per-partition
    # contiguous images, so every device DMA is a plain 2-D slice copy:
    #   xs0: all of x.T + W1's first single strip (smallest possible first
    #        chunk -> earliest PE start)
    #   w1s: W1's second single strip, then strip pairs 1..7
    #   w2s: W2 in quad-chunk layout [p][quad][k][h]
    xs0 = nc.dram_tensor("xs0", [128, KH * cap + KH * 128], mmdt,
                         kind="ExternalInput").ap()
    w1s_d = nc.dram_tensor("w1s", [128, KH * 128 + 7 * KH * 256], mmdt,
                           kind="ExternalInput").ap()
    w2s_d = nc.dram_tensor("w2s", [128, KF * HIDDEN], mmdt,
                           kind="ExternalInput").ap()
    # biases pre-swizzled on host to [128, KF] / [128, KH] (partition-major)
    b1 = nc.dram_tensor("b1", [128, KF], fp32, kind="ExternalInput").ap()
    b2 = nc.dram_tensor("b2", [128, KH], fp32, kind="ExternalInput").ap()
    yT = nc.dram_tensor("yT", [HIDDEN, cap], fp32, kind="ExternalOutput").ap()

    # Phase-1 weights stream as M-strip pairs (all K rows for two 128-wide F
    # tiles, >=512B contiguous runs per partition): a strip's matmuls finish
    # one PSUM bank, the GELU drains it, and the bank recycles — the PE tracks
    # the DMA stream with a few live banks instead of needing all 16
    # accumulators at once.

    with tile.TileContext(nc) as tc:
        with (
            tc.tile_pool(name="xt_pool", bufs=KH) as xt_pool,
            tc.tile_pool(name="w1_pool", bufs=8) as w1_pool,
            tc.tile_pool(name="w2_pool", bufs=1) as w2_pool,
            tc.tile_pool(name="bias_pool", bufs=1) as bias_pool,
            tc.tile_pool(name="ht_pool", bufs=KF) as ht_pool,
            tc.tile_pool(name="out_pool", bufs=4) as out_pool,
            tc.tile_pool(name="ps_pool", bufs=8, space="PSUM") as ps_pool,
        ):
            # x.T + W1's first single strip in ONE contiguous DMA
            xw0 = xt_pool.tile([128, KH * cap + KH * 128], mmdt, name="xw0",
                               tag="xw0", bufs=1)
            nc.sync.dma_start(xw0[:], xs0[:])
            xta = xw0[:, :KH * cap]
            strip0a = xw0[:, KH * cap:]
            strip0b = w1_pool.tile([128, KH * 128], mmdt, name="w1s0b",
                                   tag="w1s0b", bufs=1)
            nc.sync.dma_start(strip0b[:], w1s_d[:, :KH * 128])

            def load_w1_strip_pair(mp):
                t = w1_pool.tile([128, KH * 256], mmdt, name=f"w1s{mp}",
                                 tag="w1s")
                off = KH * 128 + (mp - 1) * KH * 256
                nc.sync.dma_start(t[:], w1s_d[:, off:off + KH * 256])
                return t

            # PE p-state warmup: the PE runs at half clock until ~3us of
            # continuous busy (HAM). Burn the DMA-wait window with tiny
            # matmuls on a zeroed tile so the real stream starts warm.
            wz = bias_pool.tile([128, 128], mmdt, name="wz", tag="wz")
            nc.gpsimd.memset(wz[:], 0.0)
            wps = ps_pool.tile([128, 128], fp32, name="wps", tag="ps")
            for i in range(40):
                nc.tensor.matmul(wps[:], wz[:], wz[:],
                                 start=True, stop=True)

            # ---- phase 1: hT[m] = gelu(W1.T @ xT + b1)  [F on partitions] ----
            hts = [None] * KF
            for mp in range(KF // 2):
                strip = None if mp == 0 else load_w1_strip_pair(mp)
                if mp == 0:
                    b1s = bias_pool.tile([128, KF], fp32, name="b1s", tag="b1s")
                    nc.scalar.dma_start(b1s[:], b1[:])
                    b2s = bias_pool.tile([128, KH], fp32, name="b2s", tag="b2s")
                    nc.scalar.dma_start(b2s[:], b2[:])
                for half in range(2):
                    m = 2 * mp + half
                    psb = ps_pool.tile([128, cap], fp32, name=f"ps1_{m}",
                                       tag="ps")
                    for k in range(KH):
                        if mp == 0:
                            src = strip0a if half == 0 else strip0b
                            lhsT = src[:, k * 128:(k + 1) * 128]
                        else:
                            off = k * 256 + half * 128
                            lhsT = strip[:, off:off + 128]
                        nc.tensor.matmul(
                            psb[:], lhsT,
                            xta[:, k * cap:(k + 1) * cap],
                            start=(k == 0), stop=(k == KH - 1))
                    ht = ht_pool.tile([128, cap], mmdt, name=f"ht{m}",
                                      tag="ht")
                    nc.scalar.activation(
                        ht[:], psb[:],
                        mybir.ActivationFunctionType.Gelu,
                        bias=b1s[:, m:m + 1])
                    hts[m] = ht

            # ---- phase 2: yT[m] = W2.T @ hT + b2  [H on partitions] ----
            # W2 streams in 2 MB quads during phase 1 and is fully resident
            # well before the PE reaches it. Each m-chain then runs its 16
            # matmuls back-to-back and evacuates immediately — stores stream
            # inline with the remaining chains instead of bunching at the end.
            w2qs = []
            for q in range(4):
                t = w2_pool.tile([128, 4 * HIDDEN], mmdt, name=f"w2q{q}",
                                 tag=f"w2q{q}", bufs=1)
                nc.sync.dma_start(
                    t[:], w2s_d[:, q * 4 * HIDDEN:(q + 1) * 4 * HIDDEN])
                w2qs.append(t)

            def w2_lhsT(k, m):
                q, kk = divmod(k, 4)
                off = kk * HIDDEN + m * 128
                return w2qs[q][:, off:off + 128]

            ps2 = [ps_pool.tile([128, cap], fp32, name=f"ps2_{m}", tag="ps")
                   for m in range(KH)]
            ot = None
            for m in range(KH):
                for k in range(KF):
                    nc.tensor.matmul(
                        ps2[m][:], w2_lhsT(k, m), hts[k][:],
                        start=(k == 0), stop=(k == KF - 1))
                # bias-add into an m-pair tile (DVE even m, ACT odd m) and
                # store both halves in one DMA on alternating rings
                if m >= KH - 2:
                    # final chains: single-m stores, shortest possible tail
                    os_ = out_pool.tile([128, cap], fp32,
                                        name=f"os{m}", tag=f"os{m % 2}")
                    nc.vector.tensor_scalar_add(
                        os_[:], ps2[m][:], b2s[:, m:m + 1])
                    eng = nc.scalar if m % 2 == 0 else nc.sync
                    eng.dma_start(yT[m * 128:(m + 1) * 128, :], os_[:])
                elif m % 2 == 0:
                    ot = out_pool.tile([128, 2 * cap], fp32,
                                       name=f"ot{m}", tag="ot")
                    nc.vector.tensor_scalar_add(
                        ot[:, :cap], ps2[m][:], b2s[:, m:m + 1])
                else:
                    nc.scalar.activation(
                        ot[:, cap:], ps2[m][:],
                        mybir.ActivationFunctionType.Identity,
                        bias=b2s[:, m:m + 1])
                    eng = nc.scalar if (m // 2) % 2 == 0 else nc.sync
                    eng.dma_start(
                        yT[(m - 1) * 128:(m + 1) * 128, :]
                        .rearrange("(c p) t -> p c t", p=128),
                        ot.rearrange("p (c t) -> p c t", c=2))

    _split_multi_waits(nc)
    return nc


def _get_nc(cap: int) -> bass.Bass:
    key = (cap, MM_DTYPE)
    if key not in _compiled_cache:
        _compiled_cache[key] = _build_nc(cap, MM_DTYPE)
    return _compiled_cache[key]


def _reference_numpy(x, idx, W1, b1, W2, b2):
    """Exact CPU path (erf-gelu in float64). Used only if routing is so
    imbalanced that one expert exceeds 512 tokens (breaks the device tiling)
    or the device path fails — slow but correct."""
    import math
    erf = np.vectorize(math.erf, otypes=[np.float64])
    out = np.zeros_like(x, dtype=np.float64)
    for e in range(NUM_EXPERTS):
        rows = np.nonzero(idx == e)[0]
        if rows.size == 0:
            continue
        h = x[rows].astype(np.float64) @ W1[e].astype(np.float64) + b1[e]
        h = h * 0.5 * (1.0 + erf(h / np.sqrt(2.0)))
        out[rows] = h @ W2[e].astype(np.float64) + b2[e]
    return out.astype(np.float32)


def kernel(x, expert_indices, W1, b1, W2, b2):
    x = np.ascontiguousarray(np.asarray(x, dtype=np.float32))
    idx = np.asarray(expert_indices).astype(np.int64)
    W1 = np.asarray(W1, dtype=np.float32)
    W2 = np.asarray(W2, dtype=np.float32)
    b1 = np.asarray(b1, dtype=np.float32)
    b2 = np.asarray(b2, dtype=np.float32)

    counts = np.bincount(idx, minlength=NUM_EXPERTS)
    # fp32r single-pass mode needs moving dim >= 256; one PSUM bank caps it
    # at 512 fp32. Even-align for 4B-multiple rows in every dtype.
    cap = max(256, int(-(-int(counts.max()) // 2)) * 2)
    if cap > 512:  # pathological routing, exceeds one PSUM bank
        return _reference_numpy(x, idx, W1, b1, W2, b2)
    nc = _get_nc(cap)

    # dispatch: stable sort tokens by expert
    order = np.argsort(idx, kind="stable")
    starts = np.zeros(NUM_EXPERTS + 1, dtype=np.int64)
    np.cumsum(counts, out=starts[1:])

    np_mmdt = np.float16 if MM_DTYPE == mybir.dt.float16 else np.float32
    in_maps = []
    tok_of_core = []
    for e in range(NUM_EXPERTS):
        toks = order[starts[e]:starts[e + 1]]
        tok_of_core.append(toks)
        xs = np.zeros((HIDDEN, cap), dtype=np_mmdt)
        xs[:, :len(toks)] = x[toks].T
        w1e = W1[e].astype(np_mmdt)
        xs0 = np.concatenate([
            xs.reshape(KH, 128, cap).transpose(1, 0, 2).reshape(128, -1),
            w1e[:, :128].reshape(KH, 128, 128)
            .transpose(1, 0, 2).reshape(128, -1),
        ], axis=1)
        w1s = np.concatenate([
            w1e[:, 128:256].reshape(KH, 128, 128)
            .transpose(1, 0, 2).reshape(128, -1),
            w1e[:, 256:].reshape(KH, 128, 7, 256)
            .transpose(1, 2, 0, 3).reshape(128, -1),
        ], axis=1)
        w2s = (W2[e].astype(np_mmdt).reshape(4, 4, 128, HIDDEN)
               .transpose(2, 0, 1, 3).reshape(128, -1))
        in_maps.append({
            "xs0": np.ascontiguousarray(xs0),
            "w1s": np.ascontiguousarray(w1s),
            "w2s": np.ascontiguousarray(w2s),
            "b1": np.ascontiguousarray(b1[e].reshape(KF, 128).T),
            "b2": np.ascontiguousarray(b2[e].reshape(KH, 128).T),
        })

    try:
        res = _run_spmd_cached(nc, in_maps)
    except Exception:
        try:  # transient failures recover on retry; fall back to the shim
            res = run_bass_kernel_spmd(nc, in_maps,
                                       core_ids=list(range(N_CORES)))
        except Exception:
            return _reference_numpy(x, idx, W1, b1, W2, b2)
    global LAST_RESULTS
    LAST_RESULTS = res

    out = np.zeros((TOKENS, HIDDEN), dtype=np.float32)
    for e in range(NUM_EXPERTS):
        toks = tok_of_core[e]
        out[toks] = res.results[e]["yT"][:, :len(toks)].T
    return out
